# revision 7
# baseline (speedup 1.0000x reference)
"""MetaAttention Trainium2 kernel.

Problem: B=2, T=2048, C=2048, H=16 heads (D=128), 1024-token memory bank
appended (unprojected) to K/V, causal mask over real tokens, memory tokens
attendable by every query; QKV projection + output projection.

Sharding: 8 cores = 2 batch groups x 4 head groups (4 heads each).
Per core: QKV projection for its 4 heads' columns over its batch,
attention for 4 (head) x 1 (batch), partial output projection with its
head-rows of W_proj. Host sums the 4 partials per batch.

All matmuls run in float32r (full-rate fp32 path, ~1e-4 accurate).
Layouts are chosen so no on-device transposes are needed anywhere:
  scoresT[s,t] = KT_chunk.T @ qT      (lhsT = KT chunk [d,s])
  OT[d,t]     += V_chunk.T @ expT     (lhsT = V chunk [s,d], rhs = expT[s,t])
  y[t,n]      += OT_chunk.T @ Wp      (lhsT = OT chunk [d,t])
Causal masking multiplies exp tiles by 0/1 masks (GpSimd) for the 4
diagonal s-chunks of each t-block; fully-masked chunks are skipped.
Softmax denominator: deferred DVE accumulation of exp tiles (two
accumulators), then a ones-matmul reduces partitions, reciprocal,
broadcast back via a second ones-matmul. Max-subtraction is skipped
(scores are O(5); exp is safe in fp32).
Attention processes two heads in interleaved lockstep so the tensor
engine always has an independent matmul available (keeps HAM warm).
"""

import contextlib
import os
import sys
import types

import numpy as np

for _p in ("/opt/trn_rl_repo", "/root/.axon_site/_ro/trn_rl_repo"):
    if os.path.isdir(_p) and _p not in sys.path:
        sys.path.append(_p)

import concourse.bass as bass  # noqa: E402
import concourse.mybir as mybir  # noqa: E402
import concourse.tile as tile  # noqa: E402
from concourse import bacc  # noqa: E402
from concourse.bass_utils import run_bass_kernel_spmd  # noqa: E402

F32 = mybir.dt.float32
F32R = mybir.dt.float32r
EXP = mybir.ActivationFunctionType.Exp
COPY = mybir.ActivationFunctionType.Copy

B, T, C = 2, 2048, 2048
H, D, MEM = 16, 128, 1024
HPG = 4            # heads per group (per core)
HG = H // HPG      # head groups
CORES = 8
TB = 512           # t-block width
NTB = T // TB      # 4 t-blocks
NCH = C // 128     # 16 contraction chunks
NMC = MEM // 128   # 8 memory s-chunks
SCALE = 1.0 / float(np.sqrt(D))

LAST_EXEC_TIME_NS = None
_CACHE = {}


def _install_ntff_hook():
    """Register the axon NTFF profiling hook if the image lacks antenv.axon_hooks."""
    try:
        import antenv  # noqa: PLC0415

        if "antenv.axon_hooks" in sys.modules:
            return
        mod = types.ModuleType("antenv.axon_hooks")
        state = {"hook": None}
        mod.set_axon_ntff_profile_hook = lambda h: state.__setitem__("hook", h)
        mod.get_axon_ntff_profile_hook = lambda: state["hook"]
        sys.modules["antenv.axon_hooks"] = mod
        antenv.axon_hooks = mod
        from trn_agent_boot.trn_boot import _ntff_profile_via_ctypes  # noqa: PLC0415

        mod.set_axon_ntff_profile_hook(
            _ntff_profile_via_ctypes("/opt/axon/libaxon_pjrt.so")
        )
    except Exception:
        pass


def _build(qkv_bias: bool, proj_bias: bool):
    nc = bacc.Bacc("TRN2", target_bir_lowering=False, debug=False, num_devices=CORES)

    xT_d = nc.declare_dram_parameter("xT", [NCH, 128, T], F32R, isOutput=False)
    wq_d = nc.declare_dram_parameter("wq", [NCH, 128, HPG * D], F32R, isOutput=False)
    wk_d = nc.declare_dram_parameter("wk", [NCH, 128, HPG * D], F32R, isOutput=False)
    wv_d = nc.declare_dram_parameter("wv", [NCH, 128, HPG * D], F32R, isOutput=False)
    memT_d = nc.declare_dram_parameter("memT", [HPG, 128, MEM], F32R, isOutput=False)
    memV_d = nc.declare_dram_parameter("memV", [NMC, 128, HPG * D], F32R, isOutput=False)
    wp_d = nc.declare_dram_parameter("wp", [HPG, 128, C], F32R, isOutput=False)
    masks_d = nc.declare_dram_parameter("masks", [4, 128, TB], F32, isOutput=False)
    onesc_d = nc.declare_dram_parameter("onesc", [128, 1], F32R, isOutput=False)
    ones512_d = nc.declare_dram_parameter("ones512", [1, TB], F32R, isOutput=False)
    if qkv_bias:
        bq_d = nc.declare_dram_parameter("bq", [1, HPG * D], F32R, isOutput=False)
        bk_d = nc.declare_dram_parameter("bk", [1, HPG * D], F32R, isOutput=False)
        bv_d = nc.declare_dram_parameter("bv", [1, HPG * D], F32R, isOutput=False)
    if proj_bias:
        bp_d = nc.declare_dram_parameter("bp", [1, C], F32R, isOutput=False)
    y_d = nc.declare_dram_parameter("y", [T, C], F32, isOutput=True)

    # DRAM bounce for qT (keeps SBUF under budget; reloaded per t-block).
    qs_d = nc.dram_tensor("qs", [NTB, 128, HPG * TB], F32R)

    with tile.TileContext(nc) as tc, nc.allow_low_precision(
        reason="float32r matmul inputs"
    ), contextlib.ExitStack() as ctx:
        prod = ctx.enter_context(tc.tile_pool(name="prod", bufs=1))
        const = ctx.enter_context(tc.tile_pool(name="const", bufs=1))

        kT = [prod.tile([128, T], F32R, tag=f"kT{h}", name=f"kT{h}") for h in range(HPG)]
        v_all = prod.tile([128, NCH * HPG * D], F32R)

        ones_col = const.tile([128, 1], F32R)
        nc.sync.dma_start(out=ones_col, in_=onesc_d[:])
        ones512 = const.tile([1, TB], F32R)
        nc.sync.dma_start(out=ones512, in_=ones512_d[:])
        if qkv_bias:
            bq_t = const.tile([1, HPG * D], F32R, tag="bq")
            bk_t = const.tile([1, HPG * D], F32R, tag="bk")
            bv_t = const.tile([1, HPG * D], F32R, tag="bv")
            nc.sync.dma_start(out=bq_t, in_=bq_d[:])
            nc.sync.dma_start(out=bk_t, in_=bk_d[:])
            nc.sync.dma_start(out=bv_t, in_=bv_d[:])
        if proj_bias:
            bp_t = const.tile([1, C], F32R)
            nc.sync.dma_start(out=bp_t, in_=bp_d[:])

        # ---------------- Phase 1: QKV projection ----------------
        # q pass (4 chains, bufs=2), then fused k+v pass (8 chains, bufs=1)
        # sharing one x stream. W loads split into 4 chunk-DMAs for early
        # start; memT/memV/masks prefetched during phase 1.
        big = ctx.enter_context(tc.tile_pool(name="big", bufs=1))
        with contextlib.ExitStack() as p1:
            wpool = p1.enter_context(tc.tile_pool(name="wpool", bufs=2))
            xpool = p1.enter_context(tc.tile_pool(name="xpool", bufs=2))
            stg = p1.enter_context(tc.tile_pool(name="stg", bufs=2))

            def load_w(w_dram, nm):
                wt = wpool.tile([128, NCH * HPG * D], F32R, tag="w", name=nm)
                for g in range(4):
                    nc.sync.dma_start(
                        out=wt[:, g * 4 * 512 : (g + 1) * 4 * 512]
                        .rearrange("p (i n) -> p i n", i=4),
                        in_=w_dram[4 * g : 4 * g + 4].rearrange("i p n -> p i n"),
                    )
                return wt

            def load_x(j, ib, nm):
                xt = xpool.tile([128, 4 * TB], F32R, tag="x", name=nm)
                nc.sync.dma_start(
                    out=xt.rearrange("p (i n) -> p i n", i=4),
                    in_=xT_d[4 * ib : 4 * ib + 4, :, j * TB : (j + 1) * TB]
                    .rearrange("i p n -> p i n"),
                )
                return xt

            wq_t = load_w(wq_d, "wq_t")
            wk_t = load_w(wk_d, "wk_t")

            # --- q pass ---
            qscope = contextlib.ExitStack()
            psq = qscope.enter_context(tc.tile_pool(name="psq", bufs=1, space="PSUM"))
            for j in range(NTB):
                ps = [psq.tile([128, TB], F32, tag=f"q{h}", name=f"psq{h}", bufs=2)
                      for h in range(HPG)]
                for ib in range(NCH // 4):
                    xt = load_x(j, ib, "xt")
                    for ii in range(4):
                        i = 4 * ib + ii
                        for h in range(HPG):
                            nc.tensor.matmul(
                                ps[h],
                                wq_t[:, i * 512 + h * D : i * 512 + (h + 1) * D],
                                xt[:, ii * TB : (ii + 1) * TB],
                                start=(i == 0),
                                stop=(i == NCH - 1 and not qkv_bias),
                            )
                if qkv_bias:
                    for h in range(HPG):
                        nc.tensor.matmul(
                            ps[h], bq_t[0:1, h * D : (h + 1) * D],
                            ones512, start=False, stop=True,
                        )
                for h in range(HPG):
                    st = stg.tile([128, TB], F32R, tag="st", name="st")
                    nc.scalar.activation(out=st, in_=ps[h], func=COPY, scale=1.0)
                    nc.sync.dma_start(out=qs_d[j, :, h * TB : (h + 1) * TB], in_=st)

            qscope.close()
            # prefetch attention constants while kv pass runs
            wv_t = load_w(wv_d, "wv_t")
            memT = big.tile([128, HPG * MEM], F32R, tag="memT")
            nc.sync.dma_start(
                out=memT.rearrange("p (h n) -> p h n", h=HPG),
                in_=memT_d.rearrange("h p n -> p h n"),
            )
            memV = big.tile([128, NMC * HPG * D], F32R, tag="memV")
            nc.sync.dma_start(
                out=memV.rearrange("p (c n) -> p c n", c=NMC),
                in_=memV_d.rearrange("c p n -> p c n"),
            )
            masks = big.tile([128, 4 * TB], F32, tag="masks")
            nc.sync.dma_start(
                out=masks.rearrange("p (k n) -> p k n", k=4),
                in_=masks_d.rearrange("k p n -> p k n"),
            )

            # --- fused k+v pass ---
            pskv = p1.enter_context(tc.tile_pool(name="pskv", bufs=1, space="PSUM"))
            for j in range(NTB):
                psk = [pskv.tile([128, TB], F32, tag=f"k{h}", name=f"psk{h}", bufs=1)
                       for h in range(HPG)]
                psv = [pskv.tile([128, TB], F32, tag=f"v{m}", name=f"psv{m}", bufs=1)
                       for m in range(4)]
                for ib in range(NCH // 4):
                    xt = load_x(j, ib, "xtkv")
                    for ii in range(4):
                        i = 4 * ib + ii
                        for h in range(HPG):
                            nc.tensor.matmul(
                                psk[h],
                                wk_t[:, i * 512 + h * D : i * 512 + (h + 1) * D],
                                xt[:, ii * TB : (ii + 1) * TB],
                                start=(i == 0),
                                stop=(i == NCH - 1 and not qkv_bias),
                            )
                        for m in range(4):
                            nc.tensor.matmul(
                                psv[m],
                                xt[:, ii * TB + m * 128 : ii * TB + (m + 1) * 128],
                                wv_t[:, i * 512 : (i + 1) * 512],
                                start=(i == 0),
                                stop=(i == NCH - 1 and not qkv_bias),
                            )
                if qkv_bias:
                    for h in range(HPG):
                        nc.tensor.matmul(
                            psk[h], bk_t[0:1, h * D : (h + 1) * D],
                            ones512, start=False, stop=True,
                        )
                    for m in range(4):
                        nc.tensor.matmul(
                            psv[m], ones512[0:1, 0:128], bv_t, start=False, stop=True
                        )
                for h in range(HPG):
                    nc.scalar.activation(
                        out=kT[h][:, j * TB : (j + 1) * TB],
                        in_=psk[h], func=COPY, scale=1.0,
                    )
                for m in range(4):
                    cch = j * 4 + m
                    nc.scalar.activation(
                        out=v_all[:, cch * 512 : (cch + 1) * 512],
                        in_=psv[m], func=COPY, scale=1.0,
                    )

        # ---------------- Phase 2+3: attention + projection ----------------
        with contextlib.ExitStack() as p2:
            qpool = p2.enter_context(tc.tile_pool(name="qpool", bufs=2))
            wppool = p2.enter_context(tc.tile_pool(name="wppool", bufs=2))
            epool = p2.enter_context(tc.tile_pool(name="epool", bufs=2))
            dpool = p2.enter_context(tc.tile_pool(name="dpool", bufs=2))
            otn_pool = p2.enter_context(tc.tile_pool(name="otnp", bufs=1))
            ypool = p2.enter_context(tc.tile_pool(name="ypool", bufs=2))
            ps_sc = p2.enter_context(tc.tile_pool(name="ps_sc", bufs=2, space="PSUM"))
            ps_ot = p2.enter_context(tc.tile_pool(name="ps_ot", bufs=1, space="PSUM"))
            ps_y = p2.enter_context(tc.tile_pool(name="ps_y", bufs=2, space="PSUM"))

            for j in range(NTB):
                qj = qpool.tile([128, HPG * TB], F32R, tag="qj", name="qj")
                nc.sync.dma_start(out=qj, in_=qs_d[j])
                otn = [otn_pool.tile([128, TB], F32R, tag=f"h{h}", name=f"otn{h}")
                       for h in range(HPG)]

                for pair in ((0, 1), (2, 3)):
                    # per-chunk schedule entries: (kind, c); lanes get opposite
                    # orders so one lane's mask-multiply stall is covered by
                    # the other lane's matmuls.
                    diag = [("diag", cc) for cc in range(4 * j, 4 * j + 4)]
                    real = [("real", cc) for cc in range(4 * j)]
                    mem = [("mem", cc) for cc in range(NMC)]
                    scheds = {0: diag + real + mem, 1: mem + real + diag}
                    npair = len(scheds[0]) // 2

                    ot, dn = {}, {}
                    for lane, h in enumerate(pair):
                        ot[h] = ps_ot.tile([128, TB], F32, tag=f"ot{lane}",
                                           name=f"ot{lane}")
                        dn[h] = dpool.tile([128, 2 * TB], F32R, tag=f"dn{lane}",
                                           name=f"dn{lane}")
                    for p in range(npair):
                        for lane, h in enumerate(pair):
                            sched = scheds[lane]
                            pc = (sched[2 * p], sched[2 * p + 1])
                            sc = ps_sc.tile([128, 2 * TB], F32, tag="sc", name="sc")
                            for half, (kind, cc) in enumerate(pc):
                                if kind == "mem":
                                    ktile = memT[:, h * MEM + cc * 128
                                                 : h * MEM + (cc + 1) * 128]
                                else:
                                    ktile = kT[h][:, cc * 128 : (cc + 1) * 128]
                                nc.tensor.matmul(
                                    sc[:, half * TB : (half + 1) * TB],
                                    ktile, qj[:, h * TB : (h + 1) * TB],
                                    start=True, stop=True,
                                )
                            e = epool.tile([128, 2 * TB], F32R, tag=f"e{lane}",
                                           name=f"e{lane}")
                            nc.scalar.activation(out=e, in_=sc, func=EXP, scale=SCALE)
                            if pc[0][0] == "diag":
                                k0 = pc[0][1] - 4 * j
                                nc.gpsimd.tensor_mul(
                                    out=e, in0=e,
                                    in1=masks[:, k0 * TB : (k0 + 2) * TB],
                                )
                            if p == 0:
                                nc.vector.tensor_copy(out=dn[h], in_=e)
                            else:
                                nc.vector.tensor_add(out=dn[h], in0=dn[h], in1=e)
                            for half, (kind, cc) in enumerate(pc):
                                if kind == "mem":
                                    vtile = memV[:, cc * 512 + h * D
                                                 : cc * 512 + (h + 1) * D]
                                else:
                                    vtile = v_all[:, cc * 512 + h * D
                                                  : cc * 512 + (h + 1) * D]
                                nc.tensor.matmul(
                                    ot[h], vtile, e[:, half * TB : (half + 1) * TB],
                                    start=(p == 0 and half == 0),
                                    stop=(p == npair - 1 and half == 1),
                                )
                    for lane, h in enumerate(pair):
                        dcomb = dpool.tile([128, TB], F32R, tag=f"dc{lane}",
                                           name=f"dcomb{lane}")
                        nc.vector.tensor_add(
                            out=dcomb, in0=dn[h][:, 0:TB], in1=dn[h][:, TB : 2 * TB]
                        )
                        dn_ps = ps_sc.tile([1, TB], F32, tag="sc", name="dn_ps")
                        nc.tensor.matmul(dn_ps, ones_col, dcomb, start=True, stop=True)
                        recip = dpool.tile([1, TB], F32R, tag=f"rc{lane}",
                                           name=f"recip{lane}")
                        nc.vector.reciprocal(out=recip, in_=dn_ps)
                        rb_ps = ps_sc.tile([128, TB], F32, tag="sc", name="rb_ps")
                        nc.tensor.matmul(rb_ps, ones512[0:1, 0:128], recip,
                                         start=True, stop=True)
                        rb = dpool.tile([128, TB], F32R, tag=f"rb{lane}",
                                        name=f"rb{lane}", bufs=1)
                        nc.vector.tensor_copy(out=rb, in_=rb_ps)
                        nc.vector.tensor_mul(out=otn[h], in0=ot[h], in1=rb)

                # Output projection for this t-block.
                for nb in range(C // TB):
                    wpt = wppool.tile([128, HPG * TB], F32R, tag="wp", name="wpt")
                    nc.sync.dma_start(
                        out=wpt.rearrange("p (h n) -> p h n", h=HPG),
                        in_=wp_d[:, :, nb * TB : (nb + 1) * TB]
                        .rearrange("h p n -> p h n"),
                    )
                    for m in range(4):
                        py = ps_y.tile([128, TB], F32, tag="y", name="py")
                        for h in range(HPG):
                            nc.tensor.matmul(
                                py,
                                otn[h][:, m * 128 : (m + 1) * 128],
                                wpt[:, h * TB : (h + 1) * TB],
                                start=(h == 0),
                                stop=(h == HPG - 1 and not proj_bias),
                            )
                        if proj_bias:
                            nc.tensor.matmul(
                                py, ones512[0:1, 0:128],
                                bp_t[0:1, nb * TB : (nb + 1) * TB],
                                start=False, stop=True,
                            )
                        yt = ypool.tile([128, TB], F32, tag="yt", name="yt")
                        nc.vector.tensor_copy(out=yt, in_=py)
                        nc.sync.dma_start(
                            out=y_d[j * TB + m * 128 : j * TB + (m + 1) * 128,
                                    nb * TB : (nb + 1) * TB],
                            in_=yt,
                        )

    nc.compile()
    return nc


def _prep_core_inputs(c, x, W_qkv, b_qkv, memory_bank, W_proj, b_proj,
                      masks, qkv_bias, proj_bias):
    b, hg = c // HG, c % HG
    cols = slice(512 * hg, 512 * hg + 512)
    ca = np.ascontiguousarray
    xT = ca(x[b].T.reshape(NCH, 128, T))
    m = {
        "xT": xT,
        "wq": ca(W_qkv[:, cols].reshape(NCH, 128, HPG * D)),
        "wk": ca(W_qkv[:, C:][:, cols].reshape(NCH, 128, HPG * D)),
        "wv": ca(W_qkv[:, 2 * C:][:, cols].reshape(NCH, 128, HPG * D)),
        "memT": ca(np.stack([
            memory_bank[0][:, 512 * hg + 128 * h : 512 * hg + 128 * (h + 1)].T
            for h in range(HPG)])),
        "memV": ca(memory_bank[0][:, cols].reshape(NMC, 128, HPG * D)),
        "wp": ca(W_proj[512 * hg : 512 * (hg + 1), :].reshape(HPG, 128, C)),
        "masks": masks,
        "onesc": np.ones((128, 1), np.float32),
        "ones512": np.ones((1, TB), np.float32),
    }
    if qkv_bias:
        m["bq"] = ca(b_qkv[cols].reshape(1, HPG * D))
        m["bk"] = ca(b_qkv[C:][cols].reshape(1, HPG * D))
        m["bv"] = ca(b_qkv[2 * C:][cols].reshape(1, HPG * D))
    if proj_bias:
        m["bp"] = ca((b_proj / HG).reshape(1, C).astype(np.float32))
    return m


def kernel(x, W_qkv, b_qkv, memory_bank, W_proj, b_proj):
    global LAST_EXEC_TIME_NS
    _install_ntff_hook()
    x = np.asarray(x, np.float32)
    W_qkv = np.asarray(W_qkv, np.float32)
    b_qkv = np.asarray(b_qkv, np.float32)
    memory_bank = np.asarray(memory_bank, np.float32)
    W_proj = np.asarray(W_proj, np.float32)
    b_proj = np.asarray(b_proj, np.float32)

    qkv_bias = bool(np.any(b_qkv != 0))
    proj_bias = bool(np.any(b_proj != 0))

    key = (qkv_bias, proj_bias)
    if key not in _CACHE:
        _CACHE[key] = _build(qkv_bias, proj_bias)
    nc = _CACHE[key]

    # 0/1 mask tile k: rows s = t0 + 128k + i, cols t = t0 + jj; allowed iff s <= t
    i_idx = np.arange(128)[:, None]
    jj = np.arange(TB)[None, :]
    masks = np.stack([
        (i_idx + 128 * k <= jj).astype(np.float32) for k in range(4)
    ])

    in_maps = [
        _prep_core_inputs(c, x, W_qkv, b_qkv, memory_bank, W_proj, b_proj,
                          masks, qkv_bias, proj_bias)
        for c in range(CORES)
    ]
    trace = os.environ.get("KERNEL_TRACE", "0") == "1"
    res = run_bass_kernel_spmd(nc, in_maps, list(range(CORES)), trace=trace)
    LAST_EXEC_TIME_NS = res.exec_time_ns

    out = np.empty((B, T, C), np.float32)
    for b in range(B):
        acc = res.results[b * HG]["y"].astype(np.float32)
        for g in range(1, HG):
            acc = acc + res.results[b * HG + g]["y"]
        out[b] = acc
    return out


# revision 8
# speedup vs baseline: 1.0398x; 1.0398x over previous
"""MetaAttention Trainium2 kernel.

Problem: B=2, T=2048, C=2048, H=16 heads (D=128), 1024-token memory bank
appended (unprojected) to K/V, causal mask over real tokens, memory tokens
attendable by every query; QKV projection + output projection.

Sharding: 8 cores = 2 batch groups x 4 head groups (4 heads each).
Per core: QKV projection for its 4 heads' columns over its batch,
attention for 4 (head) x 1 (batch), partial output projection with its
head-rows of W_proj. Host sums the 4 partials per batch.

All matmuls run in float32r (full-rate fp32 path, ~1e-4 accurate).
Layouts are chosen so no on-device transposes are needed anywhere:
  scoresT[s,t] = KT_chunk.T @ qT      (lhsT = KT chunk [d,s])
  OT[d,t]     += V_chunk.T @ expT     (lhsT = V chunk [s,d], rhs = expT[s,t])
  y[t,n]      += OT_chunk.T @ Wp      (lhsT = OT chunk [d,t])
Causal masking multiplies exp tiles by 0/1 masks (GpSimd) for the 4
diagonal s-chunks of each t-block; fully-masked chunks are skipped.
Softmax denominator: deferred DVE accumulation of exp tiles (two
accumulators), then a ones-matmul reduces partitions, reciprocal,
broadcast back via a second ones-matmul. Max-subtraction is skipped
(scores are O(5); exp is safe in fp32).
Attention processes two heads in interleaved lockstep so the tensor
engine always has an independent matmul available (keeps HAM warm).
"""

import contextlib
import os
import sys
import types

import numpy as np

for _p in ("/opt/trn_rl_repo", "/root/.axon_site/_ro/trn_rl_repo"):
    if os.path.isdir(_p) and _p not in sys.path:
        sys.path.append(_p)

import concourse.bass as bass  # noqa: E402
import concourse.mybir as mybir  # noqa: E402
import concourse.tile as tile  # noqa: E402
from concourse import bacc  # noqa: E402
from concourse.bass_utils import run_bass_kernel_spmd  # noqa: E402

F32 = mybir.dt.float32
F32R = mybir.dt.float32r
EXP = mybir.ActivationFunctionType.Exp
COPY = mybir.ActivationFunctionType.Copy

B, T, C = 2, 2048, 2048
H, D, MEM = 16, 128, 1024
HPG = 4            # heads per group (per core)
HG = H // HPG      # head groups
CORES = 8
TB = 512           # t-block width
NTB = T // TB      # 4 t-blocks
NCH = C // 128     # 16 contraction chunks
NMC = MEM // 128   # 8 memory s-chunks
SCALE = 1.0 / float(np.sqrt(D))

LAST_EXEC_TIME_NS = None
_CACHE = {}


def _install_ntff_hook():
    """Register the axon NTFF profiling hook if the image lacks antenv.axon_hooks."""
    try:
        import antenv  # noqa: PLC0415

        if "antenv.axon_hooks" in sys.modules:
            return
        mod = types.ModuleType("antenv.axon_hooks")
        state = {"hook": None}
        mod.set_axon_ntff_profile_hook = lambda h: state.__setitem__("hook", h)
        mod.get_axon_ntff_profile_hook = lambda: state["hook"]
        sys.modules["antenv.axon_hooks"] = mod
        antenv.axon_hooks = mod
        from trn_agent_boot.trn_boot import _ntff_profile_via_ctypes  # noqa: PLC0415

        mod.set_axon_ntff_profile_hook(
            _ntff_profile_via_ctypes("/opt/axon/libaxon_pjrt.so")
        )
    except Exception:
        pass


def _build(qkv_bias: bool, proj_bias: bool):
    nc = bacc.Bacc("TRN2", target_bir_lowering=False, debug=False, num_devices=CORES)

    xT_d = nc.declare_dram_parameter("xT", [NCH, 128, T], F32R, isOutput=False)
    wq_d = nc.declare_dram_parameter("wq", [NCH, 128, HPG * D], F32R, isOutput=False)
    wk_d = nc.declare_dram_parameter("wk", [NCH, 128, HPG * D], F32R, isOutput=False)
    wv_d = nc.declare_dram_parameter("wv", [NCH, 128, HPG * D], F32R, isOutput=False)
    memT_d = nc.declare_dram_parameter("memT", [HPG, 128, MEM], F32R, isOutput=False)
    memV_d = nc.declare_dram_parameter("memV", [NMC, 128, HPG * D], F32R, isOutput=False)
    wp_d = nc.declare_dram_parameter("wp", [HPG, 128, C], F32R, isOutput=False)
    masks_d = nc.declare_dram_parameter("masks", [4, 128, TB], F32, isOutput=False)
    onesc_d = nc.declare_dram_parameter("onesc", [128, 1], F32R, isOutput=False)
    ones512_d = nc.declare_dram_parameter("ones512", [1, TB], F32R, isOutput=False)
    if qkv_bias:
        bq_d = nc.declare_dram_parameter("bq", [1, HPG * D], F32R, isOutput=False)
        bk_d = nc.declare_dram_parameter("bk", [1, HPG * D], F32R, isOutput=False)
        bv_d = nc.declare_dram_parameter("bv", [1, HPG * D], F32R, isOutput=False)
    if proj_bias:
        bp_d = nc.declare_dram_parameter("bp", [1, C], F32R, isOutput=False)
    y_d = nc.declare_dram_parameter("y", [T, C], F32, isOutput=True)

    # DRAM bounce for qT (keeps SBUF under budget; reloaded per t-block).
    qs_d = nc.dram_tensor("qs", [NTB, 128, HPG * TB], F32R)

    with tile.TileContext(nc) as tc, nc.allow_low_precision(
        reason="float32r matmul inputs"
    ), contextlib.ExitStack() as ctx:
        prod = ctx.enter_context(tc.tile_pool(name="prod", bufs=1))
        const = ctx.enter_context(tc.tile_pool(name="const", bufs=1))

        kT = [prod.tile([128, T], F32R, tag=f"kT{h}", name=f"kT{h}") for h in range(HPG)]
        v_all = prod.tile([128, NCH * HPG * D], F32R)

        ones_col = const.tile([128, 1], F32R)
        nc.sync.dma_start(out=ones_col, in_=onesc_d[:])
        ones512 = const.tile([1, TB], F32R)
        nc.sync.dma_start(out=ones512, in_=ones512_d[:])
        if qkv_bias:
            bq_t = const.tile([1, HPG * D], F32R, tag="bq")
            bk_t = const.tile([1, HPG * D], F32R, tag="bk")
            bv_t = const.tile([1, HPG * D], F32R, tag="bv")
            nc.sync.dma_start(out=bq_t, in_=bq_d[:])
            nc.sync.dma_start(out=bk_t, in_=bk_d[:])
            nc.sync.dma_start(out=bv_t, in_=bv_d[:])
        if proj_bias:
            bp_t = const.tile([1, C], F32R)
            nc.sync.dma_start(out=bp_t, in_=bp_d[:])

        # ---------------- Phase 1: QKV projection ----------------
        # q pass (4 chains, bufs=2), then fused k+v pass (8 chains, bufs=1)
        # sharing one x stream. W loads split into 4 chunk-DMAs for early
        # start; memT/memV/masks prefetched during phase 1.
        big = ctx.enter_context(tc.tile_pool(name="big", bufs=1))
        with contextlib.ExitStack() as p1:
            wpool = p1.enter_context(tc.tile_pool(name="wpool", bufs=2))
            xpool = p1.enter_context(tc.tile_pool(name="xpool", bufs=2))
            stg = p1.enter_context(tc.tile_pool(name="stg", bufs=2))

            def load_w(w_dram, nm):
                wt = wpool.tile([128, NCH * HPG * D], F32R, tag="w", name=nm)
                for g in range(4):
                    nc.gpsimd.dma_start(
                        out=wt[:, g * 4 * 512 : (g + 1) * 4 * 512]
                        .rearrange("p (i n) -> p i n", i=4),
                        in_=w_dram[4 * g : 4 * g + 4].rearrange("i p n -> p i n"),
                    )
                return wt

            def load_x(j, ib, nm):
                xt = xpool.tile([128, 2 * TB], F32R, tag="x", name=nm, bufs=4)
                nc.sync.dma_start(
                    out=xt.rearrange("p (i n) -> p i n", i=2),
                    in_=xT_d[2 * ib : 2 * ib + 2, :, j * TB : (j + 1) * TB]
                    .rearrange("i p n -> p i n"),
                )
                return xt

            wq_t = load_w(wq_d, "wq_t")
            wk_t = load_w(wk_d, "wk_t")

            # --- q pass ---
            qscope = contextlib.ExitStack()
            psq = qscope.enter_context(tc.tile_pool(name="psq", bufs=1, space="PSUM"))
            for j in range(NTB):
                ps = [psq.tile([128, TB], F32, tag=f"q{h}", name=f"psq{h}", bufs=2)
                      for h in range(HPG)]
                for ib in range(NCH // 2):
                    xt = load_x(j, ib, "xt")
                    for ii in range(2):
                        i = 2 * ib + ii
                        for h in range(HPG):
                            nc.tensor.matmul(
                                ps[h],
                                wq_t[:, i * 512 + h * D : i * 512 + (h + 1) * D],
                                xt[:, ii * TB : (ii + 1) * TB],
                                start=(i == 0),
                                stop=(i == NCH - 1 and not qkv_bias),
                            )
                if qkv_bias:
                    for h in range(HPG):
                        nc.tensor.matmul(
                            ps[h], bq_t[0:1, h * D : (h + 1) * D],
                            ones512, start=False, stop=True,
                        )
                for h in range(HPG):
                    st = stg.tile([128, TB], F32R, tag="st", name="st")
                    nc.scalar.activation(out=st, in_=ps[h], func=COPY, scale=1.0)
                    nc.sync.dma_start(out=qs_d[j, :, h * TB : (h + 1) * TB], in_=st)

            qscope.close()
            # prefetch attention constants while kv pass runs
            wv_t = load_w(wv_d, "wv_t")
            memT = big.tile([128, HPG * MEM], F32R, tag="memT")
            nc.gpsimd.dma_start(
                out=memT.rearrange("p (h n) -> p h n", h=HPG),
                in_=memT_d.rearrange("h p n -> p h n"),
            )
            memV = big.tile([128, NMC * HPG * D], F32R, tag="memV")
            nc.gpsimd.dma_start(
                out=memV.rearrange("p (c n) -> p c n", c=NMC),
                in_=memV_d.rearrange("c p n -> p c n"),
            )
            masks = big.tile([128, 4 * TB], F32, tag="masks")
            nc.gpsimd.dma_start(
                out=masks.rearrange("p (k n) -> p k n", k=4),
                in_=masks_d.rearrange("k p n -> p k n"),
            )

            # --- fused k+v pass ---
            pskv = p1.enter_context(tc.tile_pool(name="pskv", bufs=1, space="PSUM"))
            for j in range(NTB):
                psk = [pskv.tile([128, TB], F32, tag=f"k{h}", name=f"psk{h}", bufs=1)
                       for h in range(HPG)]
                psv = [pskv.tile([128, TB], F32, tag=f"v{m}", name=f"psv{m}", bufs=1)
                       for m in range(4)]
                for ib in range(NCH // 2):
                    xt = load_x(j, ib, "xtkv")
                    for ii in range(2):
                        i = 2 * ib + ii
                        for h in range(HPG):
                            nc.tensor.matmul(
                                psk[h],
                                wk_t[:, i * 512 + h * D : i * 512 + (h + 1) * D],
                                xt[:, ii * TB : (ii + 1) * TB],
                                start=(i == 0),
                                stop=(i == NCH - 1 and not qkv_bias),
                            )
                        for m in range(4):
                            nc.tensor.matmul(
                                psv[m],
                                xt[:, ii * TB + m * 128 : ii * TB + (m + 1) * 128],
                                wv_t[:, i * 512 : (i + 1) * 512],
                                start=(i == 0),
                                stop=(i == NCH - 1 and not qkv_bias),
                            )
                if qkv_bias:
                    for h in range(HPG):
                        nc.tensor.matmul(
                            psk[h], bk_t[0:1, h * D : (h + 1) * D],
                            ones512, start=False, stop=True,
                        )
                    for m in range(4):
                        nc.tensor.matmul(
                            psv[m], ones512[0:1, 0:128], bv_t, start=False, stop=True
                        )
                for h in range(HPG):
                    nc.scalar.activation(
                        out=kT[h][:, j * TB : (j + 1) * TB],
                        in_=psk[h], func=COPY, scale=1.0,
                    )
                for m in range(4):
                    cch = j * 4 + m
                    nc.scalar.activation(
                        out=v_all[:, cch * 512 : (cch + 1) * 512],
                        in_=psv[m], func=COPY, scale=1.0,
                    )

        # ---------------- Phase 2+3: attention + projection ----------------
        with contextlib.ExitStack() as p2:
            qpool = p2.enter_context(tc.tile_pool(name="qpool", bufs=2))
            wppool = p2.enter_context(tc.tile_pool(name="wppool", bufs=2))
            epool = p2.enter_context(tc.tile_pool(name="epool", bufs=2))
            dpool = p2.enter_context(tc.tile_pool(name="dpool", bufs=2))
            otn_pool = p2.enter_context(tc.tile_pool(name="otnp", bufs=1))
            ypool = p2.enter_context(tc.tile_pool(name="ypool", bufs=2))
            ps_sc = p2.enter_context(tc.tile_pool(name="ps_sc", bufs=2, space="PSUM"))
            ps_ot = p2.enter_context(tc.tile_pool(name="ps_ot", bufs=1, space="PSUM"))
            ps_y = p2.enter_context(tc.tile_pool(name="ps_y", bufs=2, space="PSUM"))

            for j in range(NTB):
                qj = qpool.tile([128, HPG * TB], F32R, tag="qj", name="qj")
                nc.gpsimd.dma_start(out=qj, in_=qs_d[j])
                otn = [otn_pool.tile([128, TB], F32R, tag=f"h{h}", name=f"otn{h}")
                       for h in range(HPG)]

                for pair in ((0, 1), (2, 3)):
                    # per-chunk schedule entries: (kind, c); lanes get opposite
                    # orders so one lane's mask-multiply stall is covered by
                    # the other lane's matmuls.
                    diag = [("diag", cc) for cc in range(4 * j, 4 * j + 4)]
                    real = [("real", cc) for cc in range(4 * j)]
                    mem = [("mem", cc) for cc in range(NMC)]
                    scheds = {0: diag + real + mem, 1: mem + real + diag}
                    npair = len(scheds[0]) // 2

                    ot, dn = {}, {}
                    for lane, h in enumerate(pair):
                        ot[h] = ps_ot.tile([128, TB], F32, tag=f"ot{lane}",
                                           name=f"ot{lane}")
                        dn[h] = dpool.tile([128, 2 * TB], F32R, tag=f"dn{lane}",
                                           name=f"dn{lane}")
                    for p in range(npair):
                        for lane, h in enumerate(pair):
                            sched = scheds[lane]
                            pc = (sched[2 * p], sched[2 * p + 1])
                            sc = ps_sc.tile([128, 2 * TB], F32, tag="sc", name="sc")
                            for half, (kind, cc) in enumerate(pc):
                                if kind == "mem":
                                    ktile = memT[:, h * MEM + cc * 128
                                                 : h * MEM + (cc + 1) * 128]
                                else:
                                    ktile = kT[h][:, cc * 128 : (cc + 1) * 128]
                                nc.tensor.matmul(
                                    sc[:, half * TB : (half + 1) * TB],
                                    ktile, qj[:, h * TB : (h + 1) * TB],
                                    start=True, stop=True,
                                )
                            e = epool.tile([128, 2 * TB], F32R, tag=f"e{lane}",
                                           name=f"e{lane}")
                            nc.scalar.activation(out=e, in_=sc, func=EXP, scale=SCALE)
                            if pc[0][0] == "diag":
                                k0 = pc[0][1] - 4 * j
                                nc.gpsimd.tensor_mul(
                                    out=e, in0=e,
                                    in1=masks[:, k0 * TB : (k0 + 2) * TB],
                                )
                            if p == 0:
                                nc.vector.tensor_copy(out=dn[h], in_=e)
                            else:
                                nc.vector.tensor_add(out=dn[h], in0=dn[h], in1=e)
                            for half, (kind, cc) in enumerate(pc):
                                if kind == "mem":
                                    vtile = memV[:, cc * 512 + h * D
                                                 : cc * 512 + (h + 1) * D]
                                else:
                                    vtile = v_all[:, cc * 512 + h * D
                                                  : cc * 512 + (h + 1) * D]
                                nc.tensor.matmul(
                                    ot[h], vtile, e[:, half * TB : (half + 1) * TB],
                                    start=(p == 0 and half == 0),
                                    stop=(p == npair - 1 and half == 1),
                                )
                    for lane, h in enumerate(pair):
                        dcomb = dpool.tile([128, TB], F32R, tag=f"dc{lane}",
                                           name=f"dcomb{lane}")
                        nc.vector.tensor_add(
                            out=dcomb, in0=dn[h][:, 0:TB], in1=dn[h][:, TB : 2 * TB]
                        )
                        dn_ps = ps_y.tile([1, TB], F32, tag="y", name="dn_ps")
                        nc.tensor.matmul(dn_ps, ones_col, dcomb, start=True, stop=True)
                        recip = dpool.tile([1, TB], F32R, tag=f"rc{lane}",
                                           name=f"recip{lane}")
                        nc.vector.reciprocal(out=recip, in_=dn_ps)
                        rb_ps = ps_y.tile([128, TB], F32, tag="y", name="rb_ps")
                        nc.tensor.matmul(rb_ps, ones512[0:1, 0:128], recip,
                                         start=True, stop=True)
                        rb = dpool.tile([128, TB], F32R, tag=f"rb{lane}",
                                        name=f"rb{lane}", bufs=1)
                        nc.vector.tensor_copy(out=rb, in_=rb_ps)
                        nc.vector.tensor_mul(out=otn[h], in0=ot[h], in1=rb)

                # Output projection for this t-block.
                for nb in range(C // TB):
                    wpt = wppool.tile([128, HPG * TB], F32R, tag="wp", name="wpt")
                    nc.gpsimd.dma_start(
                        out=wpt.rearrange("p (h n) -> p h n", h=HPG),
                        in_=wp_d[:, :, nb * TB : (nb + 1) * TB]
                        .rearrange("h p n -> p h n"),
                    )
                    for m in range(4):
                        py = ps_y.tile([128, TB], F32, tag="y", name="py")
                        for h in range(HPG):
                            nc.tensor.matmul(
                                py,
                                otn[h][:, m * 128 : (m + 1) * 128],
                                wpt[:, h * TB : (h + 1) * TB],
                                start=(h == 0),
                                stop=(h == HPG - 1 and not proj_bias),
                            )
                        if proj_bias:
                            nc.tensor.matmul(
                                py, ones512[0:1, 0:128],
                                bp_t[0:1, nb * TB : (nb + 1) * TB],
                                start=False, stop=True,
                            )
                        yt = ypool.tile([128, TB], F32, tag="yt", name="yt")
                        nc.scalar.activation(out=yt, in_=py, func=COPY, scale=1.0)
                        nc.sync.dma_start(
                            out=y_d[j * TB + m * 128 : j * TB + (m + 1) * 128,
                                    nb * TB : (nb + 1) * TB],
                            in_=yt,
                        )

    nc.compile()
    return nc


def _prep_core_inputs(c, x, W_qkv, b_qkv, memory_bank, W_proj, b_proj,
                      masks, qkv_bias, proj_bias):
    b, hg = c // HG, c % HG
    cols = slice(512 * hg, 512 * hg + 512)
    ca = np.ascontiguousarray
    xT = ca(x[b].T.reshape(NCH, 128, T))
    m = {
        "xT": xT,
        "wq": ca(W_qkv[:, cols].reshape(NCH, 128, HPG * D)),
        "wk": ca(W_qkv[:, C:][:, cols].reshape(NCH, 128, HPG * D)),
        "wv": ca(W_qkv[:, 2 * C:][:, cols].reshape(NCH, 128, HPG * D)),
        "memT": ca(np.stack([
            memory_bank[0][:, 512 * hg + 128 * h : 512 * hg + 128 * (h + 1)].T
            for h in range(HPG)])),
        "memV": ca(memory_bank[0][:, cols].reshape(NMC, 128, HPG * D)),
        "wp": ca(W_proj[512 * hg : 512 * (hg + 1), :].reshape(HPG, 128, C)),
        "masks": masks,
        "onesc": np.ones((128, 1), np.float32),
        "ones512": np.ones((1, TB), np.float32),
    }
    if qkv_bias:
        m["bq"] = ca(b_qkv[cols].reshape(1, HPG * D))
        m["bk"] = ca(b_qkv[C:][cols].reshape(1, HPG * D))
        m["bv"] = ca(b_qkv[2 * C:][cols].reshape(1, HPG * D))
    if proj_bias:
        m["bp"] = ca((b_proj / HG).reshape(1, C).astype(np.float32))
    return m


def kernel(x, W_qkv, b_qkv, memory_bank, W_proj, b_proj):
    global LAST_EXEC_TIME_NS
    _install_ntff_hook()
    x = np.asarray(x, np.float32)
    W_qkv = np.asarray(W_qkv, np.float32)
    b_qkv = np.asarray(b_qkv, np.float32)
    memory_bank = np.asarray(memory_bank, np.float32)
    W_proj = np.asarray(W_proj, np.float32)
    b_proj = np.asarray(b_proj, np.float32)

    qkv_bias = bool(np.any(b_qkv != 0))
    proj_bias = bool(np.any(b_proj != 0))

    key = (qkv_bias, proj_bias)
    if key not in _CACHE:
        _CACHE[key] = _build(qkv_bias, proj_bias)
    nc = _CACHE[key]

    # 0/1 mask tile k: rows s = t0 + 128k + i, cols t = t0 + jj; allowed iff s <= t
    i_idx = np.arange(128)[:, None]
    jj = np.arange(TB)[None, :]
    masks = np.stack([
        (i_idx + 128 * k <= jj).astype(np.float32) for k in range(4)
    ])

    in_maps = [
        _prep_core_inputs(c, x, W_qkv, b_qkv, memory_bank, W_proj, b_proj,
                          masks, qkv_bias, proj_bias)
        for c in range(CORES)
    ]
    trace = os.environ.get("KERNEL_TRACE", "0") == "1"
    res = run_bass_kernel_spmd(nc, in_maps, list(range(CORES)), trace=trace)
    LAST_EXEC_TIME_NS = res.exec_time_ns

    out = np.empty((B, T, C), np.float32)
    for b in range(B):
        acc = res.results[b * HG]["y"].astype(np.float32)
        for g in range(1, HG):
            acc = acc + res.results[b * HG + g]["y"]
        out[b] = acc
    return out


# revision 9
# speedup vs baseline: 1.0453x; 1.0053x over previous
"""MetaAttention Trainium2 kernel.

Problem: B=2, T=2048, C=2048, H=16 heads (D=128), 1024-token memory bank
appended (unprojected) to K/V, causal mask over real tokens, memory tokens
attendable by every query; QKV projection + output projection.

Sharding: 8 cores = 2 batch groups x 4 head groups (4 heads each).
Per core: QKV projection for its 4 heads' columns over its batch,
attention for 4 (head) x 1 (batch), partial output projection with its
head-rows of W_proj. Host sums the 4 partials per batch.

All matmuls run in float32r (full-rate fp32 path, ~1e-4 accurate).
Layouts are chosen so no on-device transposes are needed anywhere:
  scoresT[s,t] = KT_chunk.T @ qT      (lhsT = KT chunk [d,s])
  OT[d,t]     += V_chunk.T @ expT     (lhsT = V chunk [s,d], rhs = expT[s,t])
  y[t,n]      += OT_chunk.T @ Wp      (lhsT = OT chunk [d,t])
Causal masking multiplies exp tiles by 0/1 masks (GpSimd) for the 4
diagonal s-chunks of each t-block; fully-masked chunks are skipped.
Softmax denominator: deferred DVE accumulation of exp tiles (two
accumulators), then a ones-matmul reduces partitions, reciprocal,
broadcast back via a second ones-matmul. Max-subtraction is skipped
(scores are O(5); exp is safe in fp32).
Attention processes two heads in interleaved lockstep so the tensor
engine always has an independent matmul available (keeps HAM warm).
"""

import contextlib
import os
import sys
import types

import numpy as np

for _p in ("/opt/trn_rl_repo", "/root/.axon_site/_ro/trn_rl_repo"):
    if os.path.isdir(_p) and _p not in sys.path:
        sys.path.append(_p)

import concourse.bass as bass  # noqa: E402
import concourse.mybir as mybir  # noqa: E402
import concourse.tile as tile  # noqa: E402
from concourse import bacc  # noqa: E402
from concourse.bass_utils import run_bass_kernel_spmd  # noqa: E402

F32 = mybir.dt.float32
F32R = mybir.dt.float32r
EXP = mybir.ActivationFunctionType.Exp
COPY = mybir.ActivationFunctionType.Copy

B, T, C = 2, 2048, 2048
H, D, MEM = 16, 128, 1024
HPG = 4            # heads per group (per core)
HG = H // HPG      # head groups
CORES = 8
TB = 512           # t-block width
NTB = T // TB      # 4 t-blocks
NCH = C // 128     # 16 contraction chunks
NMC = MEM // 128   # 8 memory s-chunks
SCALE = 1.0 / float(np.sqrt(D))

LAST_EXEC_TIME_NS = None
_CACHE = {}


def _install_ntff_hook():
    """Register the axon NTFF profiling hook if the image lacks antenv.axon_hooks."""
    try:
        import antenv  # noqa: PLC0415

        if "antenv.axon_hooks" in sys.modules:
            return
        mod = types.ModuleType("antenv.axon_hooks")
        state = {"hook": None}
        mod.set_axon_ntff_profile_hook = lambda h: state.__setitem__("hook", h)
        mod.get_axon_ntff_profile_hook = lambda: state["hook"]
        sys.modules["antenv.axon_hooks"] = mod
        antenv.axon_hooks = mod
        from trn_agent_boot.trn_boot import _ntff_profile_via_ctypes  # noqa: PLC0415

        mod.set_axon_ntff_profile_hook(
            _ntff_profile_via_ctypes("/opt/axon/libaxon_pjrt.so")
        )
    except Exception:
        pass


def _build(qkv_bias: bool, proj_bias: bool):
    nc = bacc.Bacc("TRN2", target_bir_lowering=False, debug=False, num_devices=CORES)

    xT_d = nc.declare_dram_parameter("xT", [NCH, 128, T], F32R, isOutput=False)
    wq_d = nc.declare_dram_parameter("wq", [NCH, 128, HPG * D], F32R, isOutput=False)
    wk_d = nc.declare_dram_parameter("wk", [NCH, 128, HPG * D], F32R, isOutput=False)
    wv_d = nc.declare_dram_parameter("wv", [NCH, 128, HPG * D], F32R, isOutput=False)
    memT_d = nc.declare_dram_parameter("memT", [HPG, 128, MEM], F32R, isOutput=False)
    memV_d = nc.declare_dram_parameter("memV", [NMC, 128, HPG * D], F32R, isOutput=False)
    wp_d = nc.declare_dram_parameter("wp", [HPG, 128, C], F32R, isOutput=False)
    masks_d = nc.declare_dram_parameter("masks", [4, 128, TB], F32, isOutput=False)
    onesc_d = nc.declare_dram_parameter("onesc", [128, 1], F32R, isOutput=False)
    ones512_d = nc.declare_dram_parameter("ones512", [1, TB], F32R, isOutput=False)
    if qkv_bias:
        bq_d = nc.declare_dram_parameter("bq", [1, HPG * D], F32R, isOutput=False)
        bk_d = nc.declare_dram_parameter("bk", [1, HPG * D], F32R, isOutput=False)
        bv_d = nc.declare_dram_parameter("bv", [1, HPG * D], F32R, isOutput=False)
    if proj_bias:
        bp_d = nc.declare_dram_parameter("bp", [1, C], F32R, isOutput=False)
    y_d = nc.declare_dram_parameter("y", [T, C], F32, isOutput=True)

    # DRAM bounce for qT (keeps SBUF under budget; reloaded per t-block).
    qs_d = nc.dram_tensor("qs", [NTB, 128, HPG * TB], F32R)

    with tile.TileContext(nc) as tc, nc.allow_low_precision(
        reason="float32r matmul inputs"
    ), contextlib.ExitStack() as ctx:
        prod = ctx.enter_context(tc.tile_pool(name="prod", bufs=1))
        const = ctx.enter_context(tc.tile_pool(name="const", bufs=1))

        kT = [prod.tile([128, T], F32R, tag=f"kT{h}", name=f"kT{h}") for h in range(HPG)]
        v_all = prod.tile([128, NCH * HPG * D], F32R)

        ones_col = const.tile([128, 1], F32R)
        nc.sync.dma_start(out=ones_col, in_=onesc_d[:])
        ones512 = const.tile([1, TB], F32R)
        nc.sync.dma_start(out=ones512, in_=ones512_d[:])
        if qkv_bias:
            bq_t = const.tile([1, HPG * D], F32R, tag="bq")
            bk_t = const.tile([1, HPG * D], F32R, tag="bk")
            bv_t = const.tile([1, HPG * D], F32R, tag="bv")
            nc.sync.dma_start(out=bq_t, in_=bq_d[:])
            nc.sync.dma_start(out=bk_t, in_=bk_d[:])
            nc.sync.dma_start(out=bv_t, in_=bv_d[:])
        if proj_bias:
            bp_t = const.tile([1, C], F32R)
            nc.sync.dma_start(out=bp_t, in_=bp_d[:])

        # ---------------- Phase 1: QKV projection ----------------
        # q pass (4 chains, bufs=2), then fused k+v pass (8 chains, bufs=1)
        # sharing one x stream. W loads split into 4 chunk-DMAs for early
        # start; memT/memV/masks prefetched during phase 1.
        big = ctx.enter_context(tc.tile_pool(name="big", bufs=1))
        with contextlib.ExitStack() as p1:
            wpool = p1.enter_context(tc.tile_pool(name="wpool", bufs=2))
            xpool = p1.enter_context(tc.tile_pool(name="xpool", bufs=2))
            stg = p1.enter_context(tc.tile_pool(name="stg", bufs=2))

            def load_w(w_dram, nm):
                wt = wpool.tile([128, NCH * HPG * D], F32R, tag="w", name=nm)
                for g in range(4):
                    nc.gpsimd.dma_start(
                        out=wt[:, g * 4 * 512 : (g + 1) * 4 * 512]
                        .rearrange("p (i n) -> p i n", i=4),
                        in_=w_dram[4 * g : 4 * g + 4].rearrange("i p n -> p i n"),
                    )
                return wt

            def load_x(j, ib, nm):
                xt = xpool.tile([128, 2 * TB], F32R, tag="x", name=nm, bufs=4)
                eng = nc.sync if ib % 2 == 0 else nc.gpsimd
                eng.dma_start(
                    out=xt.rearrange("p (i n) -> p i n", i=2),
                    in_=xT_d[2 * ib : 2 * ib + 2, :, j * TB : (j + 1) * TB]
                    .rearrange("i p n -> p i n"),
                )
                return xt

            wq_t = load_w(wq_d, "wq_t")
            wk_t = load_w(wk_d, "wk_t")

            # --- q pass ---
            qscope = contextlib.ExitStack()
            psq = qscope.enter_context(tc.tile_pool(name="psq", bufs=1, space="PSUM"))
            for j in range(NTB):
                ps = [psq.tile([128, TB], F32, tag=f"q{h}", name=f"psq{h}", bufs=2)
                      for h in range(HPG)]
                for ib in range(NCH // 2):
                    xt = load_x(j, ib, "xt")
                    for ii in range(2):
                        i = 2 * ib + ii
                        for h in range(HPG):
                            nc.tensor.matmul(
                                ps[h],
                                wq_t[:, i * 512 + h * D : i * 512 + (h + 1) * D],
                                xt[:, ii * TB : (ii + 1) * TB],
                                start=(i == 0),
                                stop=(i == NCH - 1 and not qkv_bias),
                            )
                if qkv_bias:
                    for h in range(HPG):
                        nc.tensor.matmul(
                            ps[h], bq_t[0:1, h * D : (h + 1) * D],
                            ones512, start=False, stop=True,
                        )
                for h in range(HPG):
                    st = stg.tile([128, TB], F32R, tag="st", name="st")
                    nc.scalar.activation(out=st, in_=ps[h], func=COPY, scale=1.0)
                    nc.sync.dma_start(out=qs_d[j, :, h * TB : (h + 1) * TB], in_=st)

            qscope.close()
            # prefetch attention constants while kv pass runs
            wv_t = load_w(wv_d, "wv_t")
            memT = big.tile([128, HPG * MEM], F32R, tag="memT")
            nc.gpsimd.dma_start(
                out=memT.rearrange("p (h n) -> p h n", h=HPG),
                in_=memT_d.rearrange("h p n -> p h n"),
            )
            memV = big.tile([128, NMC * HPG * D], F32R, tag="memV")
            nc.gpsimd.dma_start(
                out=memV.rearrange("p (c n) -> p c n", c=NMC),
                in_=memV_d.rearrange("c p n -> p c n"),
            )
            masks = big.tile([128, 4 * TB], F32, tag="masks")
            nc.gpsimd.dma_start(
                out=masks.rearrange("p (k n) -> p k n", k=4),
                in_=masks_d.rearrange("k p n -> p k n"),
            )

            # --- fused k+v pass ---
            pskv = p1.enter_context(tc.tile_pool(name="pskv", bufs=1, space="PSUM"))
            for j in range(NTB):
                psk = [pskv.tile([128, TB], F32, tag=f"k{h}", name=f"psk{h}", bufs=1)
                       for h in range(HPG)]
                psv = [pskv.tile([128, TB], F32, tag=f"v{m}", name=f"psv{m}", bufs=1)
                       for m in range(4)]
                for ib in range(NCH // 2):
                    xt = load_x(j, ib, "xtkv")
                    for ii in range(2):
                        i = 2 * ib + ii
                        for h in range(HPG):
                            nc.tensor.matmul(
                                psk[h],
                                wk_t[:, i * 512 + h * D : i * 512 + (h + 1) * D],
                                xt[:, ii * TB : (ii + 1) * TB],
                                start=(i == 0),
                                stop=(i == NCH - 1 and not qkv_bias),
                            )
                        for m in range(4):
                            nc.tensor.matmul(
                                psv[m],
                                xt[:, ii * TB + m * 128 : ii * TB + (m + 1) * 128],
                                wv_t[:, i * 512 : (i + 1) * 512],
                                start=(i == 0),
                                stop=(i == NCH - 1 and not qkv_bias),
                            )
                if qkv_bias:
                    for h in range(HPG):
                        nc.tensor.matmul(
                            psk[h], bk_t[0:1, h * D : (h + 1) * D],
                            ones512, start=False, stop=True,
                        )
                    for m in range(4):
                        nc.tensor.matmul(
                            psv[m], ones512[0:1, 0:128], bv_t, start=False, stop=True
                        )
                for h in range(HPG):
                    nc.scalar.activation(
                        out=kT[h][:, j * TB : (j + 1) * TB],
                        in_=psk[h], func=COPY, scale=1.0,
                    )
                for m in range(4):
                    cch = j * 4 + m
                    nc.scalar.activation(
                        out=v_all[:, cch * 512 : (cch + 1) * 512],
                        in_=psv[m], func=COPY, scale=1.0,
                    )

        # ---------------- Phase 2+3: attention + projection ----------------
        with contextlib.ExitStack() as p2:
            qpool = p2.enter_context(tc.tile_pool(name="qpool", bufs=2))
            wppool = p2.enter_context(tc.tile_pool(name="wppool", bufs=2))
            epool = p2.enter_context(tc.tile_pool(name="epool", bufs=2))
            dpool = p2.enter_context(tc.tile_pool(name="dpool", bufs=2))
            otn_pool = p2.enter_context(tc.tile_pool(name="otnp", bufs=1))
            ypool = p2.enter_context(tc.tile_pool(name="ypool", bufs=2))
            ps_sc = p2.enter_context(tc.tile_pool(name="ps_sc", bufs=2, space="PSUM"))
            ps_ot = p2.enter_context(tc.tile_pool(name="ps_ot", bufs=1, space="PSUM"))
            ps_y = p2.enter_context(tc.tile_pool(name="ps_y", bufs=2, space="PSUM"))

            for j in range(NTB):
                qj = qpool.tile([128, HPG * TB], F32R, tag="qj", name="qj")
                nc.gpsimd.dma_start(out=qj, in_=qs_d[j])
                otn = [otn_pool.tile([128, TB], F32R, tag=f"h{h}", name=f"otn{h}")
                       for h in range(HPG)]

                for pair in ((0, 1), (2, 3)):
                    # per-chunk schedule entries: (kind, c); lanes get opposite
                    # orders so one lane's mask-multiply stall is covered by
                    # the other lane's matmuls.
                    diag = [("diag", cc) for cc in range(4 * j, 4 * j + 4)]
                    real = [("real", cc) for cc in range(4 * j)]
                    mem = [("mem", cc) for cc in range(NMC)]
                    scheds = {0: diag + real + mem, 1: mem + real + diag}
                    npair = len(scheds[0]) // 2

                    ot, dn = {}, {}
                    for lane, h in enumerate(pair):
                        ot[h] = ps_ot.tile([128, TB], F32, tag=f"ot{lane}",
                                           name=f"ot{lane}")
                        dn[h] = dpool.tile([128, 2 * TB], F32R, tag=f"dn{lane}",
                                           name=f"dn{lane}", bufs=1)
                    for p in range(npair):
                        for lane, h in enumerate(pair):
                            sched = scheds[lane]
                            pc = (sched[2 * p], sched[2 * p + 1])
                            sc = ps_sc.tile([128, 2 * TB], F32, tag="sc", name="sc")
                            for half, (kind, cc) in enumerate(pc):
                                if kind == "mem":
                                    ktile = memT[:, h * MEM + cc * 128
                                                 : h * MEM + (cc + 1) * 128]
                                else:
                                    ktile = kT[h][:, cc * 128 : (cc + 1) * 128]
                                nc.tensor.matmul(
                                    sc[:, half * TB : (half + 1) * TB],
                                    ktile, qj[:, h * TB : (h + 1) * TB],
                                    start=True, stop=True,
                                )
                            e = epool.tile([128, 2 * TB], F32R, tag=f"e{lane}",
                                           name=f"e{lane}")
                            nc.scalar.activation(out=e, in_=sc, func=EXP, scale=SCALE)
                            if pc[0][0] == "diag":
                                k0 = pc[0][1] - 4 * j
                                nc.gpsimd.tensor_mul(
                                    out=e, in0=e,
                                    in1=masks[:, k0 * TB : (k0 + 2) * TB],
                                )
                            if p == 0:
                                nc.vector.tensor_copy(out=dn[h], in_=e)
                            else:
                                nc.vector.tensor_add(out=dn[h], in0=dn[h], in1=e)
                            for half, (kind, cc) in enumerate(pc):
                                if kind == "mem":
                                    vtile = memV[:, cc * 512 + h * D
                                                 : cc * 512 + (h + 1) * D]
                                else:
                                    vtile = v_all[:, cc * 512 + h * D
                                                  : cc * 512 + (h + 1) * D]
                                nc.tensor.matmul(
                                    ot[h], vtile, e[:, half * TB : (half + 1) * TB],
                                    start=(p == 0 and half == 0),
                                    stop=(p == npair - 1 and half == 1),
                                )
                    for lane, h in enumerate(pair):
                        # free the OT PSUM bank immediately; normalize in SBUF
                        otu = dpool.tile([128, TB], F32R, tag=f"ou{lane}",
                                         name=f"otu{lane}", bufs=1)
                        nc.vector.tensor_copy(out=otu, in_=ot[h])
                        dcomb = dpool.tile([128, TB], F32R, tag=f"dc{lane}",
                                           name=f"dcomb{lane}")
                        nc.vector.tensor_add(
                            out=dcomb, in0=dn[h][:, 0:TB], in1=dn[h][:, TB : 2 * TB]
                        )
                        dn_ps = ps_y.tile([1, TB], F32, tag="y", name="dn_ps")
                        nc.tensor.matmul(dn_ps, ones_col, dcomb, start=True, stop=True)
                        recip = dpool.tile([1, TB], F32R, tag=f"rc{lane}",
                                           name=f"recip{lane}")
                        nc.vector.reciprocal(out=recip, in_=dn_ps)
                        rb_ps = ps_y.tile([128, TB], F32, tag="y", name="rb_ps")
                        nc.tensor.matmul(rb_ps, ones512[0:1, 0:128], recip,
                                         start=True, stop=True)
                        rb = dpool.tile([128, TB], F32R, tag=f"rb{lane}",
                                        name=f"rb{lane}", bufs=1)
                        nc.vector.tensor_copy(out=rb, in_=rb_ps)
                        nc.vector.tensor_mul(out=otn[h], in0=otu, in1=rb)

                # Output projection for this t-block.
                for nb in range(C // TB):
                    wpt = wppool.tile([128, HPG * TB], F32R, tag="wp", name="wpt")
                    nc.gpsimd.dma_start(
                        out=wpt.rearrange("p (h n) -> p h n", h=HPG),
                        in_=wp_d[:, :, nb * TB : (nb + 1) * TB]
                        .rearrange("h p n -> p h n"),
                    )
                    for m in range(4):
                        py = ps_y.tile([128, TB], F32, tag="y", name="py")
                        for h in range(HPG):
                            nc.tensor.matmul(
                                py,
                                otn[h][:, m * 128 : (m + 1) * 128],
                                wpt[:, h * TB : (h + 1) * TB],
                                start=(h == 0),
                                stop=(h == HPG - 1 and not proj_bias),
                            )
                        if proj_bias:
                            nc.tensor.matmul(
                                py, ones512[0:1, 0:128],
                                bp_t[0:1, nb * TB : (nb + 1) * TB],
                                start=False, stop=True,
                            )
                        yt = ypool.tile([128, TB], F32, tag="yt", name="yt")
                        nc.scalar.activation(out=yt, in_=py, func=COPY, scale=1.0)
                        nc.sync.dma_start(
                            out=y_d[j * TB + m * 128 : j * TB + (m + 1) * 128,
                                    nb * TB : (nb + 1) * TB],
                            in_=yt,
                        )

    nc.compile()
    return nc


def _prep_core_inputs(c, x, W_qkv, b_qkv, memory_bank, W_proj, b_proj,
                      masks, qkv_bias, proj_bias):
    b, hg = c // HG, c % HG
    cols = slice(512 * hg, 512 * hg + 512)
    ca = np.ascontiguousarray
    xT = ca(x[b].T.reshape(NCH, 128, T))
    m = {
        "xT": xT,
        "wq": ca(W_qkv[:, cols].reshape(NCH, 128, HPG * D)),
        "wk": ca(W_qkv[:, C:][:, cols].reshape(NCH, 128, HPG * D)),
        "wv": ca(W_qkv[:, 2 * C:][:, cols].reshape(NCH, 128, HPG * D)),
        "memT": ca(np.stack([
            memory_bank[0][:, 512 * hg + 128 * h : 512 * hg + 128 * (h + 1)].T
            for h in range(HPG)])),
        "memV": ca(memory_bank[0][:, cols].reshape(NMC, 128, HPG * D)),
        "wp": ca(W_proj[512 * hg : 512 * (hg + 1), :].reshape(HPG, 128, C)),
        "masks": masks,
        "onesc": np.ones((128, 1), np.float32),
        "ones512": np.ones((1, TB), np.float32),
    }
    if qkv_bias:
        m["bq"] = ca(b_qkv[cols].reshape(1, HPG * D))
        m["bk"] = ca(b_qkv[C:][cols].reshape(1, HPG * D))
        m["bv"] = ca(b_qkv[2 * C:][cols].reshape(1, HPG * D))
    if proj_bias:
        m["bp"] = ca((b_proj / HG).reshape(1, C).astype(np.float32))
    return m


def kernel(x, W_qkv, b_qkv, memory_bank, W_proj, b_proj):
    global LAST_EXEC_TIME_NS
    _install_ntff_hook()
    x = np.asarray(x, np.float32)
    W_qkv = np.asarray(W_qkv, np.float32)
    b_qkv = np.asarray(b_qkv, np.float32)
    memory_bank = np.asarray(memory_bank, np.float32)
    W_proj = np.asarray(W_proj, np.float32)
    b_proj = np.asarray(b_proj, np.float32)

    qkv_bias = bool(np.any(b_qkv != 0))
    proj_bias = bool(np.any(b_proj != 0))

    key = (qkv_bias, proj_bias)
    if key not in _CACHE:
        _CACHE[key] = _build(qkv_bias, proj_bias)
    nc = _CACHE[key]

    # 0/1 mask tile k: rows s = t0 + 128k + i, cols t = t0 + jj; allowed iff s <= t
    i_idx = np.arange(128)[:, None]
    jj = np.arange(TB)[None, :]
    masks = np.stack([
        (i_idx + 128 * k <= jj).astype(np.float32) for k in range(4)
    ])

    in_maps = [
        _prep_core_inputs(c, x, W_qkv, b_qkv, memory_bank, W_proj, b_proj,
                          masks, qkv_bias, proj_bias)
        for c in range(CORES)
    ]
    trace = os.environ.get("KERNEL_TRACE", "0") == "1"
    res = run_bass_kernel_spmd(nc, in_maps, list(range(CORES)), trace=trace)
    LAST_EXEC_TIME_NS = res.exec_time_ns

    out = np.empty((B, T, C), np.float32)
    for b in range(B):
        acc = res.results[b * HG]["y"].astype(np.float32)
        for g in range(1, HG):
            acc = acc + res.results[b * HG + g]["y"]
        out[b] = acc
    return out
